# revision 1
# baseline (speedup 1.0000x reference)
"""Trainium2 Bass kernel for MultiHeadAttention with relative-position bias.

Problem: B=4, S=2048, D=256, H=8, d_k=32.
  q/k/v: [B,S,D] f32; rel_pos: [B,S,S] int32 in [0,10);
  out = softmax(QK^T/sqrt(d_k) + emb_fwd[rel_pos] + emb_bwd[rel_pos^T]) V, then out-proj.

Sharding: 8 cores = (batch b, query-half). Each core handles all 8 heads for
1024 query rows x 2048 keys. Outputs are disjoint row-blocks -> host concat.

On-device layout is fully transposed (features on partitions):
  scoresT[k,q] = sum_d KT[d,k] * QTs[d,q]   (QT pre-scaled by 1/sqrt(d_k))
  attnT = exp(scoresT + biasT)  (no max-subtraction; logits bounded)
  outT[dv,q] = sum_k V'[k,dv] * attnT[k,q]  with ones-column for denominator
  finalT[dout,q] = sum_d Wp[dout,d] * (outT/denom)[d,q]

The rel-pos bias LUT (10 entries per head per direction) is applied with a
custom DVE op: one instruction adds a *pair* of LUT entries using the
arithmetic indicator relu(2 - t^2) * (avg + halfdiff*t), t = 2*rp - (2v+1).
10 ops accumulate scores+bias per tile-head; embedding values are baked as
instruction immediates at build time.
"""

import sys

if "/opt/trn_rl_repo" not in sys.path:
    sys.path.insert(0, "/opt/trn_rl_repo")

import numpy as np

import concourse.bass as bass
import concourse.mybir as mybir
from concourse import bacc
from concourse import dve_ops
from concourse.dve_spec import Spec, Src0, Src1, C0, C1, C2, One, relu, sq, lower
from concourse.dve_uop import DveOpSpec
from concourse.tile import TileContext
from concourse.bass_utils import run_bass_kernel_spmd

B, S, D, H = 4, 2048, 256, 8
D_K = D // H
QH = S // 2          # query rows per core
N_CORES = 8
KT_TILES = S // 128  # 16 k-tiles of 128
QC = 2               # q chunks of 512
QCW = QH // QC       # 512
FP32 = mybir.dt.float32
BF16 = mybir.dt.bfloat16
FP16 = mybir.dt.float16
INT32 = mybir.dt.int32

_LUT2_OP = None


def _register_lut2_op():
    """Custom DVE op: out = Src1 + (C0 + C1*(Src0 - C2)) * relu(2 - (Src0 - C2)^2).

    With Src0 = 2*rp (pre-doubled rel_pos) and C2 = 2v+1, the relu factor is an
    exact indicator of rp in {v, v+1} (value 1.0 at t=+-1, 0 at |t|>=3), and the
    linear factor evaluates to C0-C1 at rp=v and C0+C1 at rp=v+1.
    """
    global _LUT2_OP
    if _LUT2_OP is not None:
        return _LUT2_OP
    t = Src0 - C2
    ind = relu((One + One) - sq(t))  # (One+One) is hoisted to a latch: 0 stages
    body = Src1 + (C0 + C1 * t) * ind

    def _ref(in0, in1, s0, s1, imm2):
        tt = in0.astype(np.float32) - imm2
        indr = np.maximum(2.0 - tt * tt, 0.0)
        return (in1 + (s0 + s1 * tt) * indr).astype(np.float32)

    spec = Spec(body=body, reference=_ref)
    # compute the sha so DveOp's pin check passes
    uops = lower(spec, ver="v3")
    sha3 = DveOpSpec(name="LUT2_ACC_ANT", opcode=0, uops=uops, rd1_en=True).sha("v3")
    op = dve_ops.DveOp("LUT2_ACC_ANT", spec, subdim=False, uops_sha={"v3": sha3})
    if all(o.name != op.name for o in dve_ops.OPS):
        dve_ops.OPS.append(op)
        dve_ops.CUSTOM_DVE_SPECS[op.name] = op.spec
        dve_ops._SUB_OPCODE_FOR_NAME[op.name] = (
            max(dve_ops._SUB_OPCODE_FOR_NAME.values()) + 1
        )
    _LUT2_OP = op
    return op


def _pair_params(lut):
    """lut: [10] floats -> list of (s0=avg, s1=halfdiff, imm2=2v+1) per pair."""
    out = []
    for v in range(0, 10, 2):
        a, b = float(lut[v]), float(lut[v + 1])
        out.append(((a + b) / 2.0, (b - a) / 2.0, float(2 * v + 1)))
    return out


def _build(emb_fwd, emb_bwd):
    """Build the SPMD Bass kernel with emb values baked as immediates."""
    op = _register_lut2_op()
    nc = bacc.Bacc("TRN2", target_bir_lowering=False, debug=False)

    # ---- DRAM I/O ----
    qT = nc.dram_tensor("qT", [D, QH], FP32, kind="ExternalInput").ap()
    kT = nc.dram_tensor("kT", [D, S], FP32, kind="ExternalInput").ap()
    vT = nc.dram_tensor("vT", [D, S], FP32, kind="ExternalInput").ap()
    rpF = nc.dram_tensor("rpF", [S, QH], INT32, kind="ExternalInput").ap()
    rpN = nc.dram_tensor("rpN", [S, QH], INT32, kind="ExternalInput").ap()
    wqT = nc.dram_tensor("wqT", [D, D], FP32, kind="ExternalInput").ap()  # pre-scaled
    wkT = nc.dram_tensor("wkT", [D, D], FP32, kind="ExternalInput").ap()
    wvT = nc.dram_tensor("wvT", [D, D], FP32, kind="ExternalInput").ap()
    wpT = nc.dram_tensor("wpT", [D, D], FP32, kind="ExternalInput").ap()
    bqs = nc.dram_tensor("bqs", [128, 2], FP32, kind="ExternalInput").ap()  # pre-scaled
    bks = nc.dram_tensor("bks", [128, 2], FP32, kind="ExternalInput").ap()
    bps = nc.dram_tensor("bps", [128, 2], FP32, kind="ExternalInput").ap()
    bv = nc.dram_tensor("bv", [D], FP32, kind="ExternalInput").ap()
    outT = nc.dram_tensor("outT", [D, QH], FP32, kind="ExternalOutput").ap()

    fw_pairs = [_pair_params(emb_fwd[:, h]) for h in range(H)]
    bw_pairs = [_pair_params(emb_bwd[:, h]) for h in range(H)]

    with TileContext(nc) as tc:
        _emit(nc, tc, locals(), fw_pairs, bw_pairs, op)
    nc.compile()
    return nc


def _emit(nc, tc, t, fw_pairs, bw_pairs, lut_op):
    qT, kT, vT, rpF, rpN = t["qT"], t["kT"], t["vT"], t["rpF"], t["rpN"]
    wqT, wkT, wvT, wpT = t["wqT"], t["wkT"], t["wvT"], t["wpT"]
    bqs, bks, bps, bv, outT = t["bqs"], t["bks"], t["bps"], t["bv"], t["outT"]
    Exp = mybir.ActivationFunctionType.Exp
    Ident = mybir.ActivationFunctionType.Identity
    AOT = mybir.AluOpType

    import contextlib
    ctx = contextlib.ExitStack()
    with ctx:
        singles = ctx.enter_context(tc.tile_pool(name="singles", bufs=1))
        stage = ctx.enter_context(tc.tile_pool(name="stage", bufs=3))
        rp2p = ctx.enter_context(tc.tile_pool(name="rp2", bufs=1))
        attnp = ctx.enter_context(tc.tile_pool(name="attn", bufs=2))
        up = ctx.enter_context(tc.tile_pool(name="u", bufs=3))
        psA = ctx.enter_context(tc.tile_pool(name="psA", bufs=2, space="PSUM"))
        psB = ctx.enter_context(tc.tile_pool(name="psB", bufs=2, space="PSUM"))

        # ---- load weights & biases ----
        w_sb = {}
        for name, ap in (("wq", wqT), ("wk", wkT), ("wv", wvT), ("wp", wpT)):
            for g in range(2):
                tl = singles.tile([128, D], FP32, name=f"w_{name}{g}", tag=f"w_{name}{g}")
                nc.sync.dma_start(out=tl, in_=ap[g * 128:(g + 1) * 128, :])
                w_sb[name, g] = tl
        b_sb = {}
        for name, ap in (("bq", bqs), ("bk", bks), ("bp", bps)):
            tl = singles.tile([128, 2], FP32, name=f"b_{name}", tag=f"b_{name}")
            nc.sync.dma_start(out=tl, in_=ap[:, :])
            b_sb[name] = tl
        bv_bc = singles.tile([128, D], FP32, name="bv_bc", tag="bv_bc")
        nc.sync.dma_start(
            out=bv_bc,
            in_=bass.AP(tensor=bv.tensor, offset=bv.offset, ap=[[0, 128], [1, D]]),
        )
        ones1 = singles.tile([1, 32], FP32, name="ones1", tag="ones1")
        nc.vector.memset(ones1, 1.0)

        # ---- projections (input tiles streamed from DRAM) ----
        # QTs[dout, q] (scaled), KTs[dout, k]: out = W^T.T @ xT
        QTs = [singles.tile([128, QH], FP32, name=f"QTs{g}", tag=f"QTs{g}") for g in range(2)]
        KTs = [singles.tile([128, S], FP32, name=f"KTs{g}", tag=f"KTs{g}") for g in range(2)]
        for dst, src_dram, wname, bname, width in (
            (QTs, qT, "wq", "bq", QH),
            (KTs, kT, "wk", "bk", S),
        ):
            for c0 in range(0, width, 512):
                xc = [
                    stage.tile([128, 512], FP32, name=f"xT{dg}", tag=f"xT{dg}")
                    for dg in range(2)
                ]
                for dg in range(2):
                    nc.sync.dma_start(
                        out=xc[dg],
                        in_=src_dram[dg * 128:(dg + 1) * 128, c0:c0 + 512],
                    )
                for g in range(2):  # output d-group
                    ps = psA.tile([128, 512], FP32, name="proj", tag="proj")
                    for dg in range(2):  # contraction d-group
                        nc.tensor.matmul(
                            ps,
                            w_sb[wname, dg][:, g * 128:(g + 1) * 128],
                            xc[dg],
                            start=(dg == 0),
                            stop=(dg == 1),
                        )
                    nc.scalar.activation(
                        dst[g][:, c0:c0 + 512], ps, Ident,
                        bias=b_sb[bname][:, g:g + 1],
                    )

        # V natural [k, (h,dv+ones)]: V[k, dout] = sum_d vT[d,k] * WvT[d,dout]
        V_sb = []
        for kt in range(KT_TILES):
            vt = singles.tile([128, H * 33], BF16, name=f"V{kt}", tag=f"V{kt}")
            V_sb.append(vt)
            vt3 = vt.rearrange("p (h c) -> p h c", h=H)
            nc.vector.memset(vt3[:, :, 32:33], 1.0)
            vc = [
                stage.tile([128, 128], FP32, name=f"vTc{dg}", tag=f"vTc{dg}")
                for dg in range(2)
            ]
            for dg in range(2):
                nc.sync.dma_start(
                    out=vc[dg],
                    in_=vT[dg * 128:(dg + 1) * 128, kt * 128:(kt + 1) * 128],
                )
            ps = psA.tile([128, 256], FP32, name="proj", tag="proj")
            for dg in range(2):
                nc.tensor.matmul(
                    ps,
                    vc[dg],
                    w_sb["wv", dg],
                    start=(dg == 0),
                    stop=(dg == 1),
                )
            nc.vector.scalar_tensor_tensor(
                out=vt3[:, :, 0:32],
                in0=ps.rearrange("p (h c) -> p h c", h=H),
                scalar=1.0,
                in1=bv_bc.rearrange("p (h c) -> p h c", h=H),
                op0=AOT.mult, op1=AOT.add,
            )

        # ---- rel-pos tiles: rp2 = 2*rp as bf16, per k-tile ----
        rpF2, rpN2 = [], []
        for kt in range(KT_TILES):
            for src, lst, nm in ((rpF, rpF2, "F"), (rpN, rpN2, "N")):
                raw = stage.tile([128, QH], INT32, name="rp_raw", tag="rp_raw")
                nc.sync.dma_start(out=raw, in_=src[kt * 128:(kt + 1) * 128, :])
                two = rp2p.tile([128, QH], BF16, name=f"rp2{nm}{kt}", tag=f"rp2{nm}{kt}")
                nc.vector.tensor_scalar(
                    out=two, in0=raw, scalar1=2.0, scalar2=None, op0=AOT.mult,
                )
                lst.append(two)

        # ---- main attention loop ----
        outTn = [singles.tile([128, QH], FP32, name=f"outTn{g}", tag=f"outTn{g}") for g in range(2)]
        for h in range(H):
            g, r0 = h // 4, (h % 4) * 32
            pairs = (
                [(rpF2, p) for p in fw_pairs[h]] + [(rpN2, p) for p in bw_pairs[h]]
            )
            # scores + bias + exp at full q-width (FD=1024)
            attnT = [
                attnp.tile([128, QH], BF16, name=f"attnT{kt}", tag=f"attnT{kt}",
                           bufs=1)
                for kt in range(KT_TILES)
            ]
            for kt in range(KT_TILES):
                ps = psB.tile([128, QH], FP32, name="scores", tag="scores")
                for qc in range(QC):
                    nc.tensor.matmul(
                        ps[:, qc * QCW:(qc + 1) * QCW],
                        KTs[g][r0:r0 + 32, kt * 128:(kt + 1) * 128],
                        QTs[g][r0:r0 + 32, qc * QCW:(qc + 1) * QCW],
                        start=True, stop=True,
                        tile_position=(r0, 0),
                    )
                u = up.tile([128, QH], FP16, name="u", tag="u")
                src1 = ps
                for (rps, (s0, s1, imm2)) in pairs:
                    nc.vector._custom_dve(
                        lut_op, out=u, in0=rps[kt],
                        in1=src1, s0=s0, s1=s1, imm2=imm2,
                    )
                    src1 = u
                nc.scalar.activation(attnT[kt], u, Exp)
            # attnV + normalize in 512-wide chunks
            for qc in range(QC):
                q0 = qc * QCW
                po = psA.tile([33, QCW], FP32, name="attnv", tag="attnv")
                for kt in range(KT_TILES):
                    nc.tensor.matmul(
                        po,
                        V_sb[kt][:, h * 33:(h + 1) * 33],
                        attnT[kt][:, q0:q0 + QCW],
                        start=(kt == 0), stop=(kt == KT_TILES - 1),
                    )
                # normalize: rows 0..32 / row 32
                recip = up.tile([1, QCW], FP32, name="recip", tag="recip")
                nc.vector.reciprocal(recip, po[32:33, :])
                rb = psA.tile([32, QCW], FP32, name="rbcast", tag="attnv")
                nc.tensor.matmul(rb, ones1, recip, start=True, stop=True)
                rb_sb = up.tile([32, QCW], FP32, name="rb_sb", tag="rb_sb")
                nc.scalar.copy(rb_sb, rb)
                nc.vector.tensor_tensor(
                    out=outTn[g][r0:r0 + 32, q0:q0 + QCW],
                    in0=po[0:32, :], in1=rb_sb, op=AOT.mult,
                )

        # ---- output projection: finalT[dout, q] = Wp @ outTn ----
        for g in range(2):  # dout group
            for c0 in range(0, QH, 512):
                ps = psA.tile([128, 512], FP32, name="proj", tag="proj")
                for dg in range(2):
                    nc.tensor.matmul(
                        ps,
                        w_sb["wp", dg][:, g * 128:(g + 1) * 128],
                        outTn[dg][:, c0:c0 + 512],
                        start=(dg == 0), stop=(dg == 1),
                    )
                fin = stage.tile([128, 512], FP32, name="fin", tag="fin")
                nc.scalar.activation(fin, ps, Ident, bias=b_sb["bp"][:, g:g + 1])
                nc.sync.dma_start(
                    out=outT[g * 128:(g + 1) * 128, c0:c0 + 512], in_=fin
                )


_CACHE = {}


def _get_kernel(emb_fwd, emb_bwd):
    key = (emb_fwd.tobytes(), emb_bwd.tobytes())
    if key not in _CACHE:
        _CACHE[key] = _build(np.asarray(emb_fwd), np.asarray(emb_bwd))
    return _CACHE[key]


def kernel(query, key, value, rel_pos, Wk, bk, Wv, bv, Wq, bq, Wp, bp,
           emb_fwd, emb_bwd):
    query = np.asarray(query, dtype=np.float32)
    key = np.asarray(key, dtype=np.float32)
    value = np.asarray(value, dtype=np.float32)
    rel_pos = np.asarray(rel_pos, dtype=np.int32)
    Wk, Wv, Wq, Wp = (np.asarray(w, dtype=np.float32) for w in (Wk, Wv, Wq, Wp))
    bk, bv, bq, bp = (np.asarray(v, dtype=np.float32) for v in (bk, bv, bq, bp))
    emb_fwd = np.asarray(emb_fwd, dtype=np.float32)
    emb_bwd = np.asarray(emb_bwd, dtype=np.float32)

    gamma = 1.0 / np.sqrt(np.float32(D_K))
    wqT = np.ascontiguousarray(Wq.T * gamma)
    wkT = np.ascontiguousarray(Wk.T)
    wvT = np.ascontiguousarray(Wv.T)
    wpT = np.ascontiguousarray(Wp.T)
    bqs = np.ascontiguousarray((bq * gamma).reshape(2, 128).T)
    bks = np.ascontiguousarray(bk.reshape(2, 128).T)
    bps = np.ascontiguousarray(bp.reshape(2, 128).T)

    nc = _get_kernel(emb_fwd, emb_bwd)

    in_maps = []
    for core in range(N_CORES):
        b, half = divmod(core, 2)
        qs = half * QH
        in_maps.append({
            "qT": np.ascontiguousarray(query[b, qs:qs + QH, :].T),
            "kT": np.ascontiguousarray(key[b].T),
            "vT": np.ascontiguousarray(value[b].T),
            "rpF": np.ascontiguousarray(rel_pos[b, qs:qs + QH, :].T),
            "rpN": np.ascontiguousarray(rel_pos[b][:, qs:qs + QH]),
            "wqT": wqT, "wkT": wkT, "wvT": wvT, "wpT": wpT,
            "bqs": bqs, "bks": bks, "bps": bps, "bv": bv,
        })

    global LAST_IN_MAPS
    LAST_IN_MAPS = in_maps
    res = run_bass_kernel_spmd(nc, in_maps, list(range(N_CORES)))

    out = np.empty((B, S, D), dtype=np.float32)
    for core in range(N_CORES):
        b, half = divmod(core, 2)
        qs = half * QH
        out[b, qs:qs + QH, :] = res.results[core]["outT"].T
    return out



# revision 2
# speedup vs baseline: 1.1999x; 1.1999x over previous
"""Trainium2 Bass kernel v2: MultiHeadAttention with rel-pos bias via
one-hot-plane matmuls in an (h, k16) packed layout.

Problem: B=4, S=2048, D=256, H=8, d_k=32.  8 cores = (batch, query-half);
each core: 8 heads x 1024 q x 2048 k.

Core idea: emb row 9 is zero (padding_idx), so the per-head bias
  bias[k,q] = emb_fwd[rpF[k,q],h] + emb_bwd[rpN[k,q],h]
needs only 9 one-hot planes per direction.  With scores in a packed
layout p = h*16+k16 (k-super-tiles ST of 16 rows), the bias for ALL
heads is 3 PE matmuls per (ST, q-chunk) over head-independent one-hot
planes G[(slot,k16), q], with host-built coefficient matrices
LHS[(slot,k16),(h,k16)] = emb[v_slot, h] * [k16==k16'].

Pipeline per (ST, qc=512):
  psumS = KBD_g0^T Q_g0 + KBD_g1^T Q_g1        (block-diag K, 2 mm)
        + LHS1^T G1 + LHS2^T G2 + LHS3^T G3    (bias, 3 mm)
  attn  = exp(psumS)  (ACT)                     [no max-sub; logits bounded]
  psumA += VrepA^T attn ; psumB += VrepB^T attn (dv 0-15 / 16-31 sections)
  psumD += Mden^T attn                          (denominator)
After all ST: recip(psumD) -> rb broadcast matmul -> OA = psumA * rb,
OB = psumB * rb -> out-proj with host-reordered Wp rows; bv folded into
bp on host (softmax rows sum to 1).
"""

import sys

if "/opt/trn_rl_repo" not in sys.path:
    sys.path.insert(0, "/opt/trn_rl_repo")

import numpy as np

import concourse.bass as bass
import concourse.mybir as mybir
from concourse import bacc
from concourse.tile import TileContext
from concourse.bass_utils import run_bass_kernel_spmd

B, S, D, H = 4, 2048, 256, 8
D_K = D // H
QH = S // 2
N_CORES = 8
NST = S // 16          # 128 k-super-tiles
KT_TILES = S // 128    # 16 (for rp replication DMAs: 8 STs each)
FP32 = mybir.dt.float32
FP16 = mybir.dt.float16
BF16 = mybir.dt.bfloat16

# plane slots: tile1 = F v0..7 ; tile2 = [F v8, N v0..6] ; tile3 = [N v7, N v8]
T1V = list(range(8))
T2V = [8, 0, 1, 2, 3, 4, 5, 6]
T3V = [7, 8]


def _build():
    nc = bacc.Bacc("TRN2", target_bir_lowering=False, debug=False)

    qT = nc.dram_tensor("qT", [D, QH], FP16, kind="ExternalInput").ap()
    kT = nc.dram_tensor("kT", [D, S], FP16, kind="ExternalInput").ap()
    vT = nc.dram_tensor("vT", [D, S], FP16, kind="ExternalInput").ap()
    rpF = nc.dram_tensor("rpF", [S, QH], BF16, kind="ExternalInput").ap()
    rpN = nc.dram_tensor("rpN", [S, QH], BF16, kind="ExternalInput").ap()
    wqT = nc.dram_tensor("wqT", [D, D], FP16, kind="ExternalInput").ap()
    wkT = nc.dram_tensor("wkT", [D, D], FP16, kind="ExternalInput").ap()
    wvT = nc.dram_tensor("wvT", [D, D], FP16, kind="ExternalInput").ap()
    wpA = nc.dram_tensor("wpA", [128, D], FP16, kind="ExternalInput").ap()
    wpB = nc.dram_tensor("wpB", [128, D], FP16, kind="ExternalInput").ap()
    bqs = nc.dram_tensor("bqs", [128, 2], FP32, kind="ExternalInput").ap()
    bks = nc.dram_tensor("bks", [128, 2], FP32, kind="ExternalInput").ap()
    bps = nc.dram_tensor("bps", [128, 2], FP32, kind="ExternalInput").ap()
    lhs1 = nc.dram_tensor("lhs1", [128, 128], FP16, kind="ExternalInput").ap()
    lhs2 = nc.dram_tensor("lhs2", [128, 128], FP16, kind="ExternalInput").ap()
    lhs3 = nc.dram_tensor("lhs3", [32, 128], FP16, kind="ExternalInput").ap()
    vc1 = nc.dram_tensor("vc1", [128, 1], FP32, kind="ExternalInput").ap()
    vc2 = nc.dram_tensor("vc2", [128, 1], FP32, kind="ExternalInput").ap()
    vc3 = nc.dram_tensor("vc3", [32, 1], FP32, kind="ExternalInput").ap()
    mvf = nc.dram_tensor("mvf", [128, 128], FP16, kind="ExternalInput").ap()
    rep16 = nc.dram_tensor("rep16", [16, 128], BF16, kind="ExternalInput").ap()
    rep128 = nc.dram_tensor("rep128", [128, 1024], FP16, kind="ExternalInput").ap()
    mkb0 = nc.dram_tensor("mkb0", [128, 1024], FP16, kind="ExternalInput").ap()
    mkb1 = nc.dram_tensor("mkb1", [128, 1024], FP16, kind="ExternalInput").ap()
    mden = nc.dram_tensor("mden", [128, 8], FP16, kind="ExternalInput").ap()
    lden = nc.dram_tensor("lden", [8, 128], FP32, kind="ExternalInput").ap()
    outT = nc.dram_tensor("outT", [D, QH], FP32, kind="ExternalOutput").ap()

    with TileContext(nc) as tc:
        _emit(nc, tc, locals())
    nc.compile()
    return nc


def _emit(nc, tc, t):
    qT, kT, vT, rpF, rpN = t["qT"], t["kT"], t["vT"], t["rpF"], t["rpN"]
    wqT, wkT, wvT = t["wqT"], t["wkT"], t["wvT"]
    wpA, wpB = t["wpA"], t["wpB"]
    bqs, bks, bps = t["bqs"], t["bks"], t["bps"]
    lhs1, lhs2, lhs3 = t["lhs1"], t["lhs2"], t["lhs3"]
    vc1, vc2, vc3 = t["vc1"], t["vc2"], t["vc3"]
    mvf, rep16 = t["mvf"], t["rep16"]
    rep128, mkb0, mkb1 = t["rep128"], t["mkb0"], t["mkb1"]
    mden, lden = t["mden"], t["lden"]
    outT = t["outT"]
    Exp = mybir.ActivationFunctionType.Exp
    Ident = mybir.ActivationFunctionType.Identity
    AOT = mybir.AluOpType

    import contextlib
    ctx = contextlib.ExitStack()
    with ctx:
        singles = ctx.enter_context(tc.tile_pool(name="singles", bufs=1))
        stage = ctx.enter_context(tc.tile_pool(name="stage", bufs=3))
        repp = ctx.enter_context(tc.tile_pool(name="rep", bufs=2))
        gp = ctx.enter_context(tc.tile_pool(name="g", bufs=3))
        kbdp = ctx.enter_context(tc.tile_pool(name="kbd", bufs=3))
        vrp = ctx.enter_context(tc.tile_pool(name="vr", bufs=3))
        attp = ctx.enter_context(tc.tile_pool(name="att", bufs=4))
        # PSUM budget (8 banks): psS "scores" x3 rotating + psA/psB x2qc + psDD
        psS = ctx.enter_context(tc.tile_pool(name="psS", bufs=3, space="PSUM"))
        psO = ctx.enter_context(tc.tile_pool(name="psO", bufs=1, space="PSUM"))

        # ---- constants ----
        c_sb = {}
        for name, ap, shp, dt in (
            ("lhs1", lhs1, [128, 128], FP16), ("lhs2", lhs2, [128, 128], FP16),
            ("lhs3", lhs3, [32, 128], FP16), ("vc1", vc1, [128, 1], FP32),
            ("vc2", vc2, [128, 1], FP32), ("vc3", vc3, [32, 1], FP32),
            ("mvf", mvf, [128, 128], FP16), ("rep16", rep16, [16, 128], BF16),
            ("rep128", rep128, [128, 1024], FP16),
            ("mkb0", mkb0, [128, 1024], FP16), ("mkb1", mkb1, [128, 1024], FP16),
            ("mden", mden, [128, 8], FP16), ("lden", lden, [8, 128], FP32),
            ("bqs", bqs, [128, 2], FP32), ("bks", bks, [128, 2], FP32),
            ("bps", bps, [128, 2], FP32),
        ):
            tl = singles.tile(shp, dt, name=name, tag=name)
            nc.sync.dma_start(out=tl, in_=ap)
            c_sb[name] = tl

        # ---- weights ----
        w_sb = {}
        for name, ap in (("wq", wqT), ("wk", wkT), ("wv", wvT)):
            for g in range(2):
                tl = singles.tile([128, D], FP16, name=f"w_{name}{g}", tag=f"w_{name}{g}")
                nc.sync.dma_start(out=tl, in_=ap[g * 128:(g + 1) * 128, :])
                w_sb[name, g] = tl
        for name, ap in (("wpA", wpA), ("wpB", wpB)):
            tl = singles.tile([128, D], FP16, name=name, tag=name)
            nc.sync.dma_start(out=tl, in_=ap)
            w_sb[name] = tl

        # ---- Q/K projections -> QTs [2][128,QH] fp16, KTs [2][128,S] fp16 ----
        QTs = [singles.tile([128, QH], FP16, name=f"QTs{g}", tag=f"QTs{g}") for g in range(2)]
        for dst, src_dram, wname, bname, width in (
            (QTs, qT, "wq", "bqs", QH),
        ):
            for c0 in range(0, width, 512):
                xc = [stage.tile([128, 512], FP16, name=f"xT{dg}", tag=f"xT{dg}")
                      for dg in range(2)]
                for dg in range(2):
                    nc.sync.dma_start(
                        out=xc[dg], in_=src_dram[dg * 128:(dg + 1) * 128, c0:c0 + 512])
                for g in range(2):
                    ps = psS.tile([128, 512], FP32, name="proj", tag="scores")
                    for dg in range(2):
                        nc.tensor.matmul(
                            ps, w_sb[wname, dg][:, g * 128:(g + 1) * 128], xc[dg],
                            start=(dg == 0), stop=(dg == 1))
                    nc.scalar.activation(
                        dst[g][:, c0:c0 + 512], ps, Ident,
                        bias=c_sb[bname][:, g:g + 1])



        # ---- attnV output accumulators (per qc): A, B + combined denom ----
        psA = [psO.tile([128, 512], FP32, name=f"psA{qc}", tag=f"psA{qc}") for qc in range(2)]
        psB = [psO.tile([128, 512], FP32, name=f"psB{qc}", tag=f"psB{qc}") for qc in range(2)]
        psDD = psO.tile([40, 512], FP32, name="psDD", tag="psDD")

        # ---- main loop over ktiles (rp replication granularity) and STs ----
        for kt in range(KT_TILES):
            # replicated rp tiles: [slot*16 partitions, (st8, q)]
            t1 = repp.tile([128, 8 * QH], BF16, name="t1", tag="t1")
            t2 = repp.tile([128, 8 * QH], BF16, name="t2", tag="t2")
            t3 = repp.tile([32, 8 * QH], BF16, name="t3", tag="t3")
            base = kt * 128 * QH
            src3 = lambda src: bass.AP(
                tensor=src.tensor, offset=src.offset + base,
                ap=[[QH, 16], [16 * QH, 8], [1, QH]])
            for j, (tile, p0, src) in enumerate(
                [(t1, 16 * j2, rpF) for j2 in range(8)]
                + [(t2, 0, rpF)] + [(t2, 16 + 16 * j2, rpN) for j2 in range(7)]
                + [(t3, 16 * j2, rpN) for j2 in range(2)]
            ):
                eng = (nc.sync, nc.scalar)[j % 2]
                eng.dma_start(out=tile[p0:p0 + 16, :], in_=src3(src))

            # per-ktile K/V input chunks + K-major projections [k128, dout256]
            kvc = {}
            for nm, src in (("k", kT), ("v", vT)):
                for dg in range(2):
                    cchunk = stage.tile([128, 128], FP16, name=f"{nm}c{dg}",
                                        tag=f"{nm}c{dg}")
                    nc.sync.dma_start(
                        out=cchunk,
                        in_=src[dg * 128:(dg + 1) * 128,
                                kt * 128:(kt + 1) * 128])
                    kvc[nm, dg] = cchunk
            vnat_st = []
            for st8 in range(8):
                vps = psS.tile([128, 512], FP32, name="vnatp", tag="scores")
                for dg in range(2):
                    nc.tensor.matmul(
                        vps[0:16, 0:256],
                        kvc["v", dg][:, st8 * 16:(st8 + 1) * 16],
                        w_sb["wv", dg],
                        start=(dg == 0), stop=(dg == 1))
                vt = repp.tile([16, 256], BF16, name=f"vnat{st8}", tag=f"vnat{st8}")
                nc.scalar.activation(vt, vps[0:16, 0:256], Ident)
                vnat_st.append(vt)
            ktt_ps = psS.tile([128, 512], FP32, name="ktt", tag="scores")
            for dg in range(2):
                nc.tensor.matmul(
                    ktt_ps[:, 0:256],
                    kvc["k", dg], w_sb["wk", dg],
                    start=(dg == 0), stop=(dg == 1))
            ktt = repp.tile([128, 256], FP16, name="ktt", tag="kttsb")
            nc.scalar.activation(ktt, ktt_ps[:, 0:256], Ident)
            # KBD_big[d, (st8,h,k16)] = KT[d, k] * mask(d,h), via PE replication
            kbd_big = []
            for g in range(2):
                kbb = repp.tile([128, 1024], FP16, name=f"kbdb{g}", tag=f"kbdb{g}")
                for ch in range(2):
                    kps = psS.tile([128, 512], FP32, name="kbdps", tag="scores")
                    nc.tensor.matmul(
                        kps, ktt[:, g * 128:(g + 1) * 128],
                        c_sb["rep128"][:, ch * 512:(ch + 1) * 512],
                        start=True, stop=True)
                    krep = attp.tile([128, 512], FP16, name="krep", tag="krep")
                    nc.scalar.copy(krep, kps)
                    nc.vector.tensor_tensor(
                        out=kbb[:, ch * 512:(ch + 1) * 512], in0=krep,
                        in1=c_sb[f"mkb{g}"][:, ch * 512:(ch + 1) * 512],
                        op=AOT.mult)
                kbd_big.append(kbb)

            for st8 in range(8):
                ST = kt * 8 + st8
                k0 = ST * 16
                qsl = slice(st8 * QH, (st8 + 1) * QH)
                ssl = slice(st8 * 128, (st8 + 1) * 128)

                # one-hot planes
                g1 = gp.tile([128, QH], BF16, name="g1", tag="g1")
                g2 = gp.tile([128, QH], BF16, name="g2", tag="g2")
                g3 = gp.tile([32, QH], BF16, name="g3", tag="g3")
                nc.vector.tensor_scalar(
                    out=g1, in0=t1[:, qsl], scalar1=c_sb["vc1"][:, 0:1],
                    scalar2=None, op0=AOT.is_equal)
                nc.vector.tensor_scalar(
                    out=g2, in0=t2[:, qsl], scalar1=c_sb["vc2"][:, 0:1],
                    scalar2=None, op0=AOT.is_equal)
                nc.vector.tensor_scalar(
                    out=g3, in0=t3[:, qsl], scalar1=c_sb["vc3"][:, 0:1],
                    scalar2=None, op0=AOT.is_equal)

                # KBD_g[d128, (h,k16)] = KT_g[d, k0+k16] * Mkf_g[d, (h,k16)]
                kbd = [kbd_big[g][:, st8 * 128:(st8 + 1) * 128] for g in range(2)]

                # V_rep[(rep8,k16), dv256] via replicated-column projection
                vrep_ps = psS.tile([128, 512], FP32, name="vrep", tag="scores")
                nc.tensor.matmul(
                    vrep_ps[:, 0:256], c_sb["rep16"],
                    vnat_st[st8], start=True, stop=True)
                vrep = vrp.tile([128, 256], BF16, name="vrep", tag="vrepsb")
                nc.scalar.activation(vrep, vrep_ps[:, 0:256], Ident)
                # head-masked V sections: vbd[p,(h,dv16)] = vrep[p, dvbase+dv]*Mv[p,h]
                # Wv cols pre-ordered on host: vrep cols = [A(h,dv0-15) | B(h,dv16-31)]
                vbd = []
                for sec in range(2):
                    vb = vrp.tile([128, 128], BF16, name=f"vbd{sec}", tag=f"vbd{sec}")
                    nc.vector.tensor_tensor(
                        out=vb, in0=vrep[:, sec * 128:(sec + 1) * 128],
                        in1=c_sb["mvf"], op=AOT.mult)
                    vbd.append(vb)

                for qc in range(2):
                    q0 = qc * 512
                    ps = psS.tile([128, 512], FP32, name="scores", tag="scores")
                    nc.tensor.matmul(ps, kbd[0], QTs[0][:, q0:q0 + 512],
                                     start=True, stop=False)
                    nc.tensor.matmul(ps, kbd[1], QTs[1][:, q0:q0 + 512],
                                     start=False, stop=False)
                    nc.tensor.matmul(ps, c_sb["lhs1"], g1[:, q0:q0 + 512],
                                     start=False, stop=False)
                    nc.tensor.matmul(ps, c_sb["lhs2"], g2[:, q0:q0 + 512],
                                     start=False, stop=False)
                    nc.tensor.matmul(ps, c_sb["lhs3"], g3[:, q0:q0 + 512],
                                     start=False, stop=True)
                    att = attp.tile([128, 512], BF16, name="att", tag="att")
                    nc.scalar.activation(att, ps, Exp)
                    first, last = (ST == 0), (ST == NST - 1)
                    nc.tensor.matmul(psA[qc], vbd[0], att,
                                     start=first, stop=last,
                                     skip_group_check=True)
                    nc.tensor.matmul(psB[qc], vbd[1], att,
                                     start=first, stop=last,
                                     skip_group_check=True)
                    nc.tensor.matmul(psDD[qc * 32:qc * 32 + 8, :], c_sb["mden"], att,
                                     start=first, stop=last,
                                     skip_group_check=True)

        # ---- normalize + out-projection ----
        for qc in range(2):
            recip = stage.tile([8, 512], FP32, name="recip", tag="recip")
            nc.vector.reciprocal(recip, psDD[qc * 32:qc * 32 + 8, :])
            rb = psS.tile([128, 512], FP32, name="rb", tag="scores")
            nc.tensor.matmul(rb, c_sb["lden"], recip, start=True, stop=True)
            rb_sb = attp.tile([128, 512], FP32, name="rb_sb", tag="rb_sb")
            nc.scalar.copy(rb_sb, rb)
            OA = attp.tile([128, 512], FP16, name="OA", tag="OA")
            OB = attp.tile([128, 512], FP16, name="OB", tag="OB")
            nc.vector.tensor_tensor(out=OA, in0=psA[qc], in1=rb_sb, op=AOT.mult)
            nc.vector.tensor_tensor(out=OB, in0=psB[qc], in1=rb_sb, op=AOT.mult)
            for g in range(2):
                ps = psS.tile([128, 512], FP32, name="fproj", tag="scores")
                nc.tensor.matmul(ps, w_sb["wpA"][:, g * 128:(g + 1) * 128], OA,
                                 start=True, stop=False)
                nc.tensor.matmul(ps, w_sb["wpB"][:, g * 128:(g + 1) * 128], OB,
                                 start=False, stop=True)
                fin = stage.tile([128, 512], FP32, name="fin", tag="fin")
                nc.scalar.activation(fin, ps, Ident, bias=c_sb["bps"][:, g:g + 1])
                nc.sync.dma_start(
                    out=outT[g * 128:(g + 1) * 128, qc * 512:qc * 512 + 512],
                    in_=fin)


_CACHE = {}


def _get_kernel():
    if "nc" not in _CACHE:
        _CACHE["nc"] = _build()
    return _CACHE["nc"]


def _consts(emb_fwd, emb_bwd, Wp, bp, bv):
    """Host-side constant tensors shared across cores."""
    ef = emb_fwd.astype(np.float64)
    eb = emb_bwd.astype(np.float64)
    eye16 = np.eye(16)

    def lhs_for(slots):
        # lhs[(j,k16),(h,k16')] = emb_dir[v_j, h] * [k16==k16']
        nslot = len(slots)
        out = np.zeros((nslot, 16, H, 16), np.float64)
        for j, (dirr, v) in enumerate(slots):
            e = ef if dirr == "F" else eb
            for h in range(H):
                out[j, :, h, :] = e[v, h] * eye16
        return out.reshape(nslot * 16, H * 16).astype(np.float16)

    slots1 = [("F", v) for v in T1V]
    slots2 = [("F", 8)] + [("N", v) for v in T2V[1:]]
    slots3 = [("N", v) for v in T3V]
    lhs1 = lhs_for(slots1)
    lhs2 = lhs_for(slots2)
    lhs3 = lhs_for(slots3)
    vc1 = np.array(T1V, np.float32).repeat(16).reshape(128, 1)
    vc2 = np.array(T2V, np.float32).repeat(16).reshape(128, 1)
    vc3 = np.array(T3V, np.float32).repeat(16).reshape(32, 1)
    didx = np.arange(128)
    mk0 = (didx[:, None] // 32 == np.arange(8)[None, :]).astype(np.float16)
    mk1 = ((didx[:, None] + 128) // 32 == np.arange(8)[None, :]).astype(np.float16)
    mden = (didx[:, None] // 16 == np.arange(8)[None, :]).astype(np.float16)
    lden = mden.T.astype(np.float32).copy()
    import ml_dtypes
    rep16c = np.tile(np.eye(16), (1, 8)).astype(ml_dtypes.bfloat16)
    # rep128[k, (st,h,k16)] = 1[k == st*16 + k16]
    karr = np.arange(128)
    st_i = np.arange(1024) // 128
    k16_i = np.arange(1024) % 16
    rep128c = (karr[:, None] == (st_i * 16 + k16_i)[None, :]).astype(np.float16)
    h_i = (np.arange(1024) // 16) % 8
    mkb0c = (karr[:, None] // 32 == h_i[None, :]).astype(np.float16)
    mkb1c = ((karr[:, None] + 128) // 32 == h_i[None, :]).astype(np.float16)
    mkf0 = np.repeat(mk0, 16, axis=1)
    mkf1 = np.repeat(mk1, 16, axis=1)
    mvf = np.repeat(mden, 16, axis=1)
    # out-proj: Wp rows reordered to (h, dv) A/B sections; bv folded into bp
    WpT = Wp.T.astype(np.float64)  # [dfull, dout]
    rowsA = np.concatenate([np.arange(h * 32, h * 32 + 16) for h in range(H)])
    rowsB = np.concatenate([np.arange(h * 32 + 16, h * 32 + 32) for h in range(H)])
    wpA = WpT[rowsA].astype(np.float16)
    wpB = WpT[rowsB].astype(np.float16)
    bps2 = (bp.astype(np.float64) + Wp.astype(np.float64) @ bv.astype(np.float64))
    bps = np.ascontiguousarray(bps2.reshape(2, 128).T.astype(np.float32))
    return dict(lhs1=lhs1, lhs2=lhs2, lhs3=lhs3, vc1=vc1, vc2=vc2, vc3=vc3,
                mvf=mvf, rep16=rep16c,
                rep128=rep128c, mkb0=mkb0c, mkb1=mkb1c, mden=mden, lden=lden,
                wpA=wpA, wpB=wpB, bps=bps)


def kernel(query, key, value, rel_pos, Wk, bk, Wv, bv, Wq, bq, Wp, bp,
           emb_fwd, emb_bwd):
    query = np.asarray(query, dtype=np.float32)
    key = np.asarray(key, dtype=np.float32)
    value = np.asarray(value, dtype=np.float32)
    rel_pos = np.asarray(rel_pos, dtype=np.int32)
    Wk, Wv, Wq, Wp = (np.asarray(w, dtype=np.float32) for w in (Wk, Wv, Wq, Wp))
    bk, bv, bq, bp = (np.asarray(v, dtype=np.float32) for v in (bk, bv, bq, bp))
    emb_fwd = np.asarray(emb_fwd, dtype=np.float32)
    emb_bwd = np.asarray(emb_bwd, dtype=np.float32)

    gamma = 1.0 / np.sqrt(np.float32(D_K))
    wqT = np.ascontiguousarray((Wq.T * gamma).astype(np.float16))
    wkT = np.ascontiguousarray(Wk.T.astype(np.float16))
    rowsA = np.concatenate([np.arange(h * 32, h * 32 + 16) for h in range(H)])
    rowsB = np.concatenate([np.arange(h * 32 + 16, h * 32 + 32) for h in range(H)])
    wvT = np.ascontiguousarray(Wv.T.astype(np.float16)[:, np.concatenate([rowsA, rowsB])])
    bqs = np.ascontiguousarray((bq * gamma).reshape(2, 128).T)
    bks = np.ascontiguousarray(bk.reshape(2, 128).T)

    consts = _consts(emb_fwd, emb_bwd, Wp, bp, bv)
    nc = _get_kernel()

    import ml_dtypes
    rp_bf = rel_pos.astype(ml_dtypes.bfloat16)

    in_maps = []
    for core in range(N_CORES):
        b, half = divmod(core, 2)
        qs = half * QH
        m = {
            "qT": np.ascontiguousarray(query[b, qs:qs + QH, :].T.astype(np.float16)),
            "kT": np.ascontiguousarray(key[b].T.astype(np.float16)),
            "vT": np.ascontiguousarray(value[b].T.astype(np.float16)),
            "rpF": np.ascontiguousarray(rp_bf[b, qs:qs + QH, :].T),
            "rpN": np.ascontiguousarray(rp_bf[b][:, qs:qs + QH]),
            "wqT": wqT, "wkT": wkT, "wvT": wvT,
            "bqs": bqs, "bks": bks,
        }
        m.update(consts)
        in_maps.append(m)

    global LAST_IN_MAPS
    LAST_IN_MAPS = in_maps
    res = run_bass_kernel_spmd(nc, in_maps, list(range(N_CORES)))

    out = np.empty((B, S, D), dtype=np.float32)
    for core in range(N_CORES):
        b, half = divmod(core, 2)
        qs = half * QH
        out[b, qs:qs + QH, :] = res.results[core]["outT"].T
    return out
